# revision 26
# baseline (speedup 1.0000x reference)
"""Trainium2 Bass kernel for nn_Attention_62362925138174.

Reference computation (per batch b, with xf = x[b].reshape(C, N), N = H*W):
    q = Wq @ xf            [8,  N]
    k = Wk @ xf            [8,  N]
    v = Wv @ xf            [C,  N]
    score[n, m] = q[:, n] . k[:, m]
    P = softmax(score, axis=n)          (per-column softmax)
    out[c, m] = sum_n v[c, n] P[n, m]
    att = gamma * out + xf

Kernel strategy (8 cores = 4 batches x 2 column-halves of N):
  - Score via the rank-8 q^T k form with 4x PE row tiling: q and k are
    projected on device into partition groups {0,32,64,96} (one matmul with
    zero-padded replicated weights), so each 128-row score tile issues four
    concurrent 512-column matmuls on distinct 32-row PE groups.
  - exp() is split between ScalarE (exact activation) and VectorE using a
    Schraudolph fast-exp: bits16 = trunc(A*s + B) written as int16 is the
    bf16 bit pattern of ~exp(s) (+-3%, zero-mean after calibrating B; the
    softmax normalization cancels the scale, final rel err ~4e-5).
  - V@E accumulates with a ones-row appended to V^T so one PSUM chain gives
    both gamma*(V @ E) (gamma folded into Wv on the host) and colsum(E).
  - 1/colsum via exp(-ln(colsum)) on ScalarE; broadcast on GpSimd; residual
    add uses the exact f32 input.
"""

import numpy as np

import concourse.bass as bass
import concourse.bacc as bacc
import concourse.tile as tile
from concourse import mybir
from concourse.bass_utils import run_bass_kernel_spmd

# Problem shape (hardcoded per contract).
B, C, H, W = 4, 64, 64, 64
N = H * W           # 4096
MHALF = N // 2      # 2048 columns of the score/output handled per core
NT = N // 128       # 32 row-tiles of the score matrix
N_CORES = 8

F32 = mybir.dt.float32
BF16 = mybir.dt.bfloat16
I16 = mybir.dt.int16
_NP_BF16 = mybir.dt.np(BF16)

# Schraudolph fast-exp constants for bf16 bit patterns (DVE truncates on
# f32->int16 convert; B calibrated offline on the actual score distribution).
FEXP_A = 184.6650390625   # 2^7 / ln 2
FEXP_B = 16249.5

_PROGRAM = None


def _scalar_owns_exp(t: int, h: int) -> bool:
    """Split the exp tiles between ScalarE (h=0) and VectorE (h=1) so the
    two engines always run concurrently within an iteration."""
    return h == 0


def _build_program() -> bass.Bass:
    nc = bacc.Bacc()

    xfp_d = nc.declare_dram_parameter("xfp", [C, N], BF16, isOutput=False)
    xkp_d = nc.declare_dram_parameter("xkp", [C, MHALF], BF16, isOutput=False)
    xkf_d = nc.declare_dram_parameter("xkf", [C, MHALF], F32, isOutput=False)
    # packed weights: wq4 [64,128] | wk4 [64,128] | wv^T*gamma [64,64]
    wpk_d = nc.declare_dram_parameter("wpk", [C, 320], BF16, isOutput=False)
    out_d = nc.declare_dram_parameter("out", [C, MHALF], F32, isOutput=True)

    EXP = mybir.ActivationFunctionType.Exp
    LN = mybir.ActivationFunctionType.Ln
    MULT = mybir.AluOpType.mult
    ADD = mybir.AluOpType.add

    from concourse.hw_specs import get_activation_tables

    act_sets = list(get_activation_tables(nc.m.arch))
    nle_id = act_sets.index("natural_log_exp_and_others")

    with TileCtx(nc) as (tc, sing, epool, apool, psS, psO):
        # ---- input loads: few large DMAs (each DMA trigger costs ~600ns of
        # queue time); weights+xkp on the scalar queue feed the k/q
        # projections first, bulk xfp/xkf on the sync queue ----
        wpk_sb = sing.tile([128, 320], BF16, name="wpk_sb")
        xkp_sb = sing.tile([128, MHALF], BF16, name="xkp_sb")
        for g in range(2):
            nc.scalar.dma_start(out=wpk_sb[64 * g : 64 * g + 64, :], in_=wpk_d[:, :])
        for g in range(2):
            nc.scalar.dma_start(out=xkp_sb[64 * g : 64 * g + 64, :], in_=xkp_d[:, :])
        wq4_sb = wpk_sb[:, 0:128]
        wk4_sb = wpk_sb[:, 128:256]
        wv_sb = wpk_sb[:, 256:320]
        xfp_sb = sing.tile([128, N], BF16, name="xfp_sb")
        nc.sync.dma_start(out=xfp_sb[0:64, :], in_=xfp_d[:, :])
        nc.sync.dma_start(out=xfp_sb[64:128, :], in_=xfp_d[:, :])
        xkf_sb = sing.tile([C, MHALF], F32, name="xkf_sb")
        nc.sync.dma_start(out=xkf_sb, in_=xkf_d[:, :])
        # activation tables are first needed by exp(0); emit the load after
        # the DMA triggers so it does not delay the weight/xkp transfers
        nc.scalar.add_instruction(
            mybir.InstLoadActFuncSet(
                name=nc.get_next_instruction_name(),
                act_func_set_id=nle_id,
                ins=[],
                outs=[],
            )
        )

        # ---- k4 = Wk-projection of this core's half, replicated in the
        # partition groups by the zero-padded wk4 weight layout ----
        k4_sb = sing.tile([128, MHALF], BF16, name="k4_sb")
        for i in range(2):
            kp = psS.tile([128, 1024], F32, tag="S", name="kp")
            for cc in range(2):
                g = cc
                lo = i * 1024 + cc * 512
                nc.tensor.matmul(
                    kp[:, cc * 512 : (cc + 1) * 512],
                    lhsT=wk4_sb[64 * g : 64 * g + 64, :],
                    rhs=xkp_sb[64 * g : 64 * g + 64, lo : lo + 512],
                    start=True,
                    stop=True,
                    tile_position=(64 * g, 0),
                )
            if i == 0:
                nc.scalar.copy(out=k4_sb[:, 0:1024], in_=kp)
            else:
                nc.vector.tensor_copy(out=k4_sb[:, 1024:2048], in_=kp)

        # ---- q4 = Wq-projection of all N columns, same replica layout ----
        # chunks 0-1 use partition group 0 (fed by the first xfp DMA) so the
        # main loop can start before the second xfp replica lands
        q4_sb = sing.tile([128, N], BF16, name="q4_sb")
        for qi in range(4):
            qp = psS.tile([128, 1024], F32, tag="S", name="qp")
            g = qi // 2
            for cc in range(2):
                lo = qi * 1024 + cc * 512
                nc.tensor.matmul(
                    qp[:, cc * 512 : (cc + 1) * 512],
                    lhsT=wq4_sb[64 * g : 64 * g + 64, :],
                    rhs=xfp_sb[64 * g : 64 * g + 64, lo : lo + 512],
                    start=True,
                    stop=True,
                    tile_position=(64 * g, 0),
                )
            qsl = slice(qi * 1024, (qi + 1) * 1024)
            if qi % 2 == 0:
                nc.scalar.copy(out=q4_sb[:, qsl], in_=qp)
            else:
                nc.vector.tensor_copy(out=q4_sb[:, qsl], in_=qp)

        # ---- vaugT[n, 0:64] = (gamma*Wv @ xf)^T tile, vaugT[n, 64] = 1 ----
        vaug_sb = sing.tile([128, NT * 65], BF16, name="vaug_sb")
        vaug3 = vaug_sb.rearrange("p (t u) -> p t u", u=65)
        nc.vector.memset(vaug3[:, :, 64:65], 1.0)

        def emit_vt_chunk(vv):
            # full-array matmuls (both xfp replicas x replicated wv/2 sum to
            # the exact projection) so in-loop chunks cost no config switch
            vtp = psS.tile([128, 512], F32, tag="S", name="vtp")
            for i in range(8):
                t = vv * 8 + i
                nc.tensor.matmul(
                    vtp[:, i * 64 : (i + 1) * 64],
                    lhsT=xfp_sb[:, t * 128 : (t + 1) * 128],
                    rhs=wv_sb,
                    start=True,
                    stop=True,
                )
            nc.vector.tensor_copy(
                out=vaug3[:, vv * 8 : (vv + 1) * 8, 0:64],
                in_=vtp.rearrange("p (i u) -> p i u", u=64),
            )

        # ---- main loop, software-pipelined: emit score(t) and exp(t), then
        # V@E(t-1), so the PE streams score(t)+V@E(t-1) back-to-back while
        # the Scalar/Vector engines exp() the previous tile pair ----
        O_ps = psO.tile([65, MHALF], F32, name="O_ps")

        def emit_score_exp(t):
            Es = []
            for h in range(2):
                S = psS.tile([128, 1024], F32, tag="S", name="S_ps")
                for cc in range(2):
                    r = 2 * h + cc
                    # full-array matmul: k4 is zero outside rows 0:8, so the
                    # 128-partition contraction picks out q-group 0 exactly
                    # (alternating PE tile configs costs ~400ns per switch)
                    nc.tensor.matmul(
                        S[:, cc * 512 : (cc + 1) * 512],
                        lhsT=q4_sb[:, t * 128 : (t + 1) * 128],
                        rhs=k4_sb[:, r * 512 : (r + 1) * 512],
                        start=True,
                        stop=True,
                    )
                if _scalar_owns_exp(t, h):
                    E = epool.tile([128, 1024], BF16, tag="E", name="E_sb")
                    nc.scalar.activation(out=E, in_=S, func=EXP)
                    Es.append(E)
                else:
                    E = epool.tile([128, 1024], I16, tag="E", name="Ei_sb")
                    nc.vector.tensor_scalar(E, S, FEXP_A, FEXP_B, MULT, ADD)
                    Es.append(E.bitcast(BF16))
            return Es

        def emit_ve(t, Es):
            va_t = vaug3[:, t, :]
            for h in range(2):
                for cc in range(2):
                    r = 2 * h + cc
                    nc.tensor.matmul(
                        O_ps[:, r * 512 : (r + 1) * 512],
                        lhsT=va_t,
                        rhs=Es[h][:, cc * 512 : (cc + 1) * 512],
                        start=(t == 0),
                        stop=(t == NT - 1),
                    )

        prev_Es = emit_score_exp(0)
        emit_vt_chunk(0)
        for t in range(1, NT):
            Es = emit_score_exp(t)
            if t in (1, 3, 5):
                emit_vt_chunk((t + 1) // 2)
            emit_ve(t - 1, prev_Es)
            prev_Es = Es
        emit_ve(NT - 1, prev_Es)

        # ---- normalize + residual, store. Tile serializes same-PSUM-tile
        # readers in EMISSION order, so emit all O_ps colsum reads (LN) before
        # the first O_ps[0:C] read (MUL); EXP/broadcast interleave between ----
        lnts, rcps, bcss = [], [], []
        for j in range(4):
            sl = slice(j * 512, (j + 1) * 512)
            lnt = apool.tile([1, 512], F32, tag="lnt", name="lnt")
            nc.scalar.activation(out=lnt, in_=O_ps[64:65, sl], func=LN)
            lnts.append(lnt)
        for j in range(4):
            rcp = apool.tile([1, 512], BF16, tag="rcp", name="rcp")
            nc.scalar.activation(out=rcp, in_=lnts[j], func=EXP, scale=-1.0)
            rcps.append(rcp)
            bcs = apool.tile([C, 512], BF16, tag="bcs", name="bcs")
            nc.gpsimd.partition_broadcast(bcs, rcp)
            bcss.append(bcs)
        for j in range(4):
            sl = slice(j * 512, (j + 1) * 512)
            tmp = apool.tile([C, 512], F32, tag="tmp", name="tmp")
            nc.vector.tensor_mul(tmp, O_ps[0:C, sl], bcss[j])
            att = apool.tile([C, 512], F32, tag="att", name="att")
            nc.vector.tensor_add(att, tmp, xkf_sb[:, sl])
            nc.sync.dma_start(out=out_d[:, sl], in_=att)

    nc.finalize()
    return nc


class TileCtx:
    """TileContext plus the tile pools used by the kernel."""

    def __init__(self, nc: bass.Bass):
        self.nc = nc

    def __enter__(self):
        from contextlib import ExitStack

        self._stack = ExitStack()
        tc = self._stack.enter_context(tile.TileContext(self.nc))
        sing = self._stack.enter_context(tc.tile_pool(name="sing", bufs=1))
        epool = self._stack.enter_context(tc.tile_pool(name="epool", bufs=6))
        apool = self._stack.enter_context(tc.tile_pool(name="apool", bufs=4))
        psS = self._stack.enter_context(tc.tile_pool(name="psS", bufs=2, space="PSUM"))
        psO = self._stack.enter_context(tc.tile_pool(name="psO", bufs=1, space="PSUM"))
        return tc, sing, epool, apool, psS, psO

    def __exit__(self, *exc):
        return self._stack.__exit__(*exc)


def get_program() -> bass.Bass:
    global _PROGRAM
    if _PROGRAM is None:
        _PROGRAM = _build_program()
    return _PROGRAM


def make_in_maps(x, Wq, Wk, Wv, gamma):
    """Shard the full inputs into per-core input maps (host-side prep only:
    reshape/slice, replicated zero-padded weight layouts, cast to bf16)."""
    x = np.ascontiguousarray(np.asarray(x, dtype=np.float32))
    Wq = np.asarray(Wq, dtype=np.float32)
    Wk = np.asarray(Wk, dtype=np.float32)
    Wv = np.asarray(Wv, dtype=np.float32)
    gamma = float(np.asarray(gamma, dtype=np.float32).reshape(()))

    def rep4(Wm):  # [8, 64] -> [64, 128] with W^T at free-cols 32a..32a+8
        out = np.zeros((C, 128), dtype=_NP_BF16)
        for a in range(4):
            out[:, 32 * a : 32 * a + 8] = Wm.T.astype(_NP_BF16)
        return out

    def rep1(Wm):  # [8, 64] -> [64, 128] with W^T only at free-cols 0..8
        out = np.zeros((C, 128), dtype=_NP_BF16)
        out[:, 0:8] = Wm.T.astype(_NP_BF16)
        return out

    # wv is halved: the v-projection contracts over both 64-row xfp replicas
    wpk = np.concatenate(
        [rep4(Wq), rep1(Wk), (0.5 * gamma * Wv.T).astype(_NP_BF16)], axis=1
    )  # [64, 320]
    wpk = np.ascontiguousarray(wpk)

    in_maps = []
    for core in range(N_CORES):
        b, h = divmod(core, 2)
        xf = x[b].reshape(C, N)
        xk = xf[:, h * MHALF : (h + 1) * MHALF]
        in_maps.append(
            {
                "xfp": xf.astype(_NP_BF16),
                "xkp": np.ascontiguousarray(xk.astype(_NP_BF16)),
                "xkf": np.ascontiguousarray(xk),
                "wpk": wpk,
            }
        )
    return in_maps


def gather(results):
    out = np.empty((B, C, N), dtype=np.float32)
    for core in range(N_CORES):
        b, h = divmod(core, 2)
        out[b][:, h * MHALF : (h + 1) * MHALF] = results[core]["out"]
    return out.reshape(B, C, H, W)


def run(inputs, **spmd_kwargs):
    nc = get_program()
    in_maps = make_in_maps(
        inputs["x"], inputs["Wq"], inputs["Wk"], inputs["Wv"], inputs["gamma"]
    )
    res = run_bass_kernel_spmd(nc, in_maps, core_ids=list(range(N_CORES)), **spmd_kwargs)
    return gather(res.results), res


def kernel(x, Wq, Wk, Wv, gamma):
    out, _ = run({"x": x, "Wq": Wq, "Wk": Wk, "Wv": Wv, "gamma": gamma})
    return out


# revision 33
# speedup vs baseline: 1.0022x; 1.0022x over previous
"""Trainium2 Bass kernel for nn_Attention_62362925138174.

Reference computation (per batch b, with xf = x[b].reshape(C, N), N = H*W):
    q = Wq @ xf            [8,  N]
    k = Wk @ xf            [8,  N]
    v = Wv @ xf            [C,  N]
    score[n, m] = q[:, n] . k[:, m]
    P = softmax(score, axis=n)          (per-column softmax)
    out[c, m] = sum_n v[c, n] P[n, m]
    att = gamma * out + xf

Kernel strategy (8 cores = 4 batches x 2 column-halves of N):
  - Score via the rank-8 q^T k form with 4x PE row tiling: q and k are
    projected on device into partition groups {0,32,64,96} (one matmul with
    zero-padded replicated weights), so each 128-row score tile issues four
    concurrent 512-column matmuls on distinct 32-row PE groups.
  - exp() is split between ScalarE (exact activation) and VectorE using a
    Schraudolph fast-exp: bits16 = trunc(A*s + B) written as int16 is the
    bf16 bit pattern of ~exp(s) (+-3%, zero-mean after calibrating B; the
    softmax normalization cancels the scale, final rel err ~4e-5).
  - V@E accumulates with a ones-row appended to V^T so one PSUM chain gives
    both gamma*(V @ E) (gamma folded into Wv on the host) and colsum(E).
  - 1/colsum via exp(-ln(colsum)) on ScalarE; broadcast on GpSimd; residual
    add uses the exact f32 input.
"""

import numpy as np

import concourse.bass as bass
import concourse.bacc as bacc
import concourse.tile as tile
from concourse import mybir
from concourse.bass_utils import run_bass_kernel_spmd

# Problem shape (hardcoded per contract).
B, C, H, W = 4, 64, 64, 64
N = H * W           # 4096
MHALF = N // 2      # 2048 columns of the score/output handled per core
NT = N // 128       # 32 row-tiles of the score matrix
N_CORES = 8

F32 = mybir.dt.float32
BF16 = mybir.dt.bfloat16
I16 = mybir.dt.int16
_NP_BF16 = mybir.dt.np(BF16)

# Schraudolph fast-exp constants for bf16 bit patterns (DVE truncates on
# f32->int16 convert; B calibrated offline on the actual score distribution).
FEXP_A = 184.6650390625   # 2^7 / ln 2
FEXP_B = 16249.5

_PROGRAM = None


def _scalar_owns_exp(t: int, h: int) -> bool:
    """Split the exp tiles between ScalarE (h=0) and VectorE (h=1) so the
    two engines always run concurrently within an iteration."""
    return h == 0


def _build_program() -> bass.Bass:
    nc = bacc.Bacc()

    xfp_d = nc.declare_dram_parameter("xfp", [C, N], BF16, isOutput=False)
    xkp_d = nc.declare_dram_parameter("xkp", [C, MHALF], BF16, isOutput=False)
    xkf_d = nc.declare_dram_parameter("xkf", [C, MHALF], F32, isOutput=False)
    # packed weights: wq4 [64,128] | wk4 [64,128] | wv^T*gamma/2 [64,64];
    # wpk2 is the partition 64-127 replica with wq4/wk4 zeroed so full-array
    # projection matmuls contract exactly once
    wpk_d = nc.declare_dram_parameter("wpk", [C, 320], BF16, isOutput=False)
    wpk2_d = nc.declare_dram_parameter("wpk2", [C, 320], BF16, isOutput=False)
    out_d = nc.declare_dram_parameter("out", [C, MHALF], F32, isOutput=True)

    EXP = mybir.ActivationFunctionType.Exp
    LN = mybir.ActivationFunctionType.Ln
    MULT = mybir.AluOpType.mult
    ADD = mybir.AluOpType.add

    from concourse.hw_specs import get_activation_tables

    act_sets = list(get_activation_tables(nc.m.arch))
    nle_id = act_sets.index("natural_log_exp_and_others")

    with TileCtx(nc) as (tc, sing, epool, apool, psS, psO):
        # ---- input loads: few large DMAs (each DMA trigger costs ~600ns of
        # queue time); weights+xkp on the scalar queue feed the k/q
        # projections first, bulk xfp/xkf on the sync queue ----
        wpk_sb = sing.tile([128, 320], BF16, name="wpk_sb")
        xkp_sb = sing.tile([C, MHALF], BF16, name="xkp_sb")
        nc.scalar.dma_start(out=wpk_sb[0:64, :], in_=wpk_d[:, :])
        nc.scalar.dma_start(out=wpk_sb[64:128, :], in_=wpk2_d[:, :])
        nc.scalar.dma_start(out=xkp_sb, in_=xkp_d[:, :])
        wq4_sb = wpk_sb[:, 0:128]
        wk4_sb = wpk_sb[:, 128:256]
        wv_sb = wpk_sb[:, 256:320]
        xfp_sb = sing.tile([128, N], BF16, name="xfp_sb")
        nc.sync.dma_start(out=xfp_sb[0:64, :], in_=xfp_d[:, :])
        nc.sync.dma_start(out=xfp_sb[64:128, :], in_=xfp_d[:, :])
        xkf_sb = sing.tile([C, MHALF], F32, name="xkf_sb")
        # activation tables are first needed by exp(0); emit the load after
        # the DMA triggers so it does not delay the weight/xkp transfers
        nc.scalar.add_instruction(
            mybir.InstLoadActFuncSet(
                name=nc.get_next_instruction_name(),
                act_func_set_id=nle_id,
                ins=[],
                outs=[],
            )
        )

        # ---- k4 = Wk-projection of this core's half (partition group 0;
        # k4 rows 8+ are zero via the rep1 weight layout) ----
        k4_sb = sing.tile([128, MHALF], BF16, name="k4_sb")
        for i in range(2):
            kp = psS.tile([128, 1024], F32, tag="S", name="kp")
            for cc in range(2):
                lo = i * 1024 + cc * 512
                nc.tensor.matmul(
                    kp[:, cc * 512 : (cc + 1) * 512],
                    lhsT=wk4_sb[0:64, :],
                    rhs=xkp_sb[:, lo : lo + 512],
                    start=True,
                    stop=True,
                    tile_position=(0, 0),
                )
            if i == 0:
                nc.scalar.copy(out=k4_sb[:, 0:1024], in_=kp)
            else:
                nc.vector.tensor_copy(out=k4_sb[:, 1024:2048], in_=kp)

        # ---- q4 = Wq-projection: chunk 0 in the prologue (group 0, fed by
        # the first xfp DMA); chunks 1-3 emitted inside the loop as
        # full-array matmuls (wq4 rows 64-127 zero), needed from t = 8*qi ----
        q4_sb = sing.tile([128, N], BF16, name="q4_sb")

        def emit_q_chunk(qi):
            qp = psS.tile([128, 1024], F32, tag="S", name="qp")
            for cc in range(2):
                lo = qi * 1024 + cc * 512
                if qi == 0:
                    nc.tensor.matmul(
                        qp[:, cc * 512 : (cc + 1) * 512],
                        lhsT=wq4_sb[0:64, :],
                        rhs=xfp_sb[0:64, lo : lo + 512],
                        start=True,
                        stop=True,
                        tile_position=(0, 0),
                    )
                else:
                    nc.tensor.matmul(
                        qp[:, cc * 512 : (cc + 1) * 512],
                        lhsT=wq4_sb,
                        rhs=xfp_sb[:, lo : lo + 512],
                        start=True,
                        stop=True,
                    )
            qsl = slice(qi * 1024, (qi + 1) * 1024)
            if qi % 2 == 0:
                nc.scalar.copy(out=q4_sb[:, qsl], in_=qp)
            else:
                nc.vector.tensor_copy(out=q4_sb[:, qsl], in_=qp)

        emit_q_chunk(0)

        # ---- vaugT[n, 0:64] = (gamma*Wv @ xf)^T tile, vaugT[n, 64] = 1 ----
        vaug_sb = sing.tile([128, NT * 65], BF16, name="vaug_sb")
        vaug3 = vaug_sb.rearrange("p (t u) -> p t u", u=65)
        nc.vector.memset(vaug3[:, :, 64:65], 1.0)

        def emit_vt_chunk(vv):
            # chunk 0: group-0 matmuls on wv/2 with a x2 copy (first xfp DMA
            # only); later chunks: full-array (both replicas x wv/2 sum
            # exactly), costing no PE config switch inside the loop
            vtp = psS.tile([128, 512], F32, tag="S", name="vtp")
            for i in range(8):
                t = vv * 8 + i
                if vv == 0:
                    nc.tensor.matmul(
                        vtp[:, i * 64 : (i + 1) * 64],
                        lhsT=xfp_sb[0:64, t * 128 : (t + 1) * 128],
                        rhs=wv_sb[0:64, :],
                        start=True,
                        stop=True,
                        tile_position=(0, 0),
                    )
                else:
                    nc.tensor.matmul(
                        vtp[:, i * 64 : (i + 1) * 64],
                        lhsT=xfp_sb[:, t * 128 : (t + 1) * 128],
                        rhs=wv_sb,
                        start=True,
                        stop=True,
                    )
            dst = vaug3[:, vv * 8 : (vv + 1) * 8, 0:64]
            src = vtp.rearrange("p (i u) -> p i u", u=64)
            if vv == 0:
                nc.vector.tensor_scalar_mul(dst, src, 2.0)
            else:
                nc.vector.tensor_copy(out=dst, in_=src)

        # ---- main loop, software-pipelined: emit score(t) and exp(t), then
        # V@E(t-1), so the PE streams score(t)+V@E(t-1) back-to-back while
        # the Scalar/Vector engines exp() the previous tile pair ----
        O_ps = psO.tile([65, MHALF], F32, name="O_ps")

        def emit_score_exp(t):
            Es = []
            for h in range(2):
                S = psS.tile([128, 1024], F32, tag="S", name="S_ps")
                for cc in range(2):
                    r = 2 * h + cc
                    # full-array matmul: k4 is zero outside rows 0:8, so the
                    # 128-partition contraction picks out q-group 0 exactly
                    # (alternating PE tile configs costs ~400ns per switch)
                    nc.tensor.matmul(
                        S[:, cc * 512 : (cc + 1) * 512],
                        lhsT=q4_sb[:, t * 128 : (t + 1) * 128],
                        rhs=k4_sb[:, r * 512 : (r + 1) * 512],
                        start=True,
                        stop=True,
                    )
                if _scalar_owns_exp(t, h):
                    E = epool.tile([128, 1024], BF16, tag="E", name="E_sb")
                    nc.scalar.activation(out=E, in_=S, func=EXP)
                    Es.append(E)
                else:
                    E = epool.tile([128, 1024], I16, tag="E", name="Ei_sb")
                    nc.vector.tensor_scalar(E, S, FEXP_A, FEXP_B, MULT, ADD)
                    Es.append(E.bitcast(BF16))
            return Es

        def emit_ve(t, Es):
            va_t = vaug3[:, t, :]
            for h in range(2):
                for cc in range(2):
                    r = 2 * h + cc
                    nc.tensor.matmul(
                        O_ps[:, r * 512 : (r + 1) * 512],
                        lhsT=va_t,
                        rhs=Es[h][:, cc * 512 : (cc + 1) * 512],
                        start=(t == 0),
                        stop=(t == NT - 1),
                    )

        prev_Es = emit_score_exp(0)
        emit_vt_chunk(0)
        for t in range(1, NT):
            Es = emit_score_exp(t)
            if t in (1, 3, 5):
                emit_q_chunk((t + 1) // 2)
            if t in (2, 4, 6):
                emit_vt_chunk(t // 2)
            if t == 8:
                # residual input is only needed by the tail
                nc.sync.dma_start(out=xkf_sb, in_=xkf_d[:, :])
            emit_ve(t - 1, prev_Es)
            prev_Es = Es
        emit_ve(NT - 1, prev_Es)

        # ---- normalize + residual, store. Tile serializes same-PSUM-tile
        # readers in EMISSION order, so emit all O_ps colsum reads (LN) before
        # the first O_ps[0:C] read (MUL); EXP/broadcast interleave between ----
        lnts, rcps, bcss = [], [], []
        for j in range(4):
            sl = slice(j * 512, (j + 1) * 512)
            lnt = apool.tile([1, 512], F32, tag="lnt", name="lnt")
            nc.scalar.activation(out=lnt, in_=O_ps[64:65, sl], func=LN)
            lnts.append(lnt)
        for j in range(4):
            rcp = apool.tile([1, 512], BF16, tag="rcp", name="rcp")
            nc.scalar.activation(out=rcp, in_=lnts[j], func=EXP, scale=-1.0)
            rcps.append(rcp)
            bcs = apool.tile([C, 512], BF16, tag="bcs", name="bcs")
            nc.gpsimd.partition_broadcast(bcs, rcp)
            bcss.append(bcs)
        for j in range(4):
            sl = slice(j * 512, (j + 1) * 512)
            tmp = apool.tile([C, 512], F32, tag="tmp", name="tmp")
            nc.vector.tensor_mul(tmp, O_ps[0:C, sl], bcss[j])
            att = apool.tile([C, 512], F32, tag="att", name="att")
            nc.vector.tensor_add(att, tmp, xkf_sb[:, sl])
            nc.sync.dma_start(out=out_d[:, sl], in_=att)

    nc.finalize()
    return nc


class TileCtx:
    """TileContext plus the tile pools used by the kernel."""

    def __init__(self, nc: bass.Bass):
        self.nc = nc

    def __enter__(self):
        from contextlib import ExitStack

        self._stack = ExitStack()
        tc = self._stack.enter_context(tile.TileContext(self.nc))
        sing = self._stack.enter_context(tc.tile_pool(name="sing", bufs=1))
        epool = self._stack.enter_context(tc.tile_pool(name="epool", bufs=6))
        apool = self._stack.enter_context(tc.tile_pool(name="apool", bufs=4))
        psS = self._stack.enter_context(tc.tile_pool(name="psS", bufs=2, space="PSUM"))
        psO = self._stack.enter_context(tc.tile_pool(name="psO", bufs=1, space="PSUM"))
        return tc, sing, epool, apool, psS, psO

    def __exit__(self, *exc):
        return self._stack.__exit__(*exc)


def get_program() -> bass.Bass:
    global _PROGRAM
    if _PROGRAM is None:
        _PROGRAM = _build_program()
    return _PROGRAM


def make_in_maps(x, Wq, Wk, Wv, gamma):
    """Shard the full inputs into per-core input maps (host-side prep only:
    reshape/slice, replicated zero-padded weight layouts, cast to bf16)."""
    x = np.ascontiguousarray(np.asarray(x, dtype=np.float32))
    Wq = np.asarray(Wq, dtype=np.float32)
    Wk = np.asarray(Wk, dtype=np.float32)
    Wv = np.asarray(Wv, dtype=np.float32)
    gamma = float(np.asarray(gamma, dtype=np.float32).reshape(()))

    def rep4(Wm):  # [8, 64] -> [64, 128] with W^T at free-cols 32a..32a+8
        out = np.zeros((C, 128), dtype=_NP_BF16)
        for a in range(4):
            out[:, 32 * a : 32 * a + 8] = Wm.T.astype(_NP_BF16)
        return out

    def rep1(Wm):  # [8, 64] -> [64, 128] with W^T only at free-cols 0..8
        out = np.zeros((C, 128), dtype=_NP_BF16)
        out[:, 0:8] = Wm.T.astype(_NP_BF16)
        return out

    # wv is halved: the v-projection contracts over both 64-row xfp replicas
    wvh = (0.5 * gamma * Wv.T).astype(_NP_BF16)
    wpk = np.ascontiguousarray(
        np.concatenate([rep4(Wq), rep1(Wk), wvh], axis=1)
    )  # [64, 320]
    wpk2 = np.ascontiguousarray(
        np.concatenate([np.zeros((C, 256), dtype=_NP_BF16), wvh], axis=1)
    )

    in_maps = []
    for core in range(N_CORES):
        b, h = divmod(core, 2)
        xf = x[b].reshape(C, N)
        xk = xf[:, h * MHALF : (h + 1) * MHALF]
        in_maps.append(
            {
                "xfp": xf.astype(_NP_BF16),
                "xkp": np.ascontiguousarray(xk.astype(_NP_BF16)),
                "xkf": np.ascontiguousarray(xk),
                "wpk": wpk,
                "wpk2": wpk2,
            }
        )
    return in_maps


def gather(results):
    out = np.empty((B, C, N), dtype=np.float32)
    for core in range(N_CORES):
        b, h = divmod(core, 2)
        out[b][:, h * MHALF : (h + 1) * MHALF] = results[core]["out"]
    return out.reshape(B, C, H, W)


def run(inputs, **spmd_kwargs):
    nc = get_program()
    in_maps = make_in_maps(
        inputs["x"], inputs["Wq"], inputs["Wk"], inputs["Wv"], inputs["gamma"]
    )
    res = run_bass_kernel_spmd(nc, in_maps, core_ids=list(range(N_CORES)), **spmd_kwargs)
    return gather(res.results), res


def kernel(x, Wq, Wk, Wv, gamma):
    out, _ = run({"x": x, "Wq": Wq, "Wk": Wk, "Wv": Wv, "gamma": gamma})
    return out


# revision 38
# speedup vs baseline: 1.0800x; 1.0775x over previous
"""Trainium2 Bass kernel for nn_Attention_62362925138174.

Reference computation (per batch b, with xf = x[b].reshape(C, N), N = H*W):
    q = Wq @ xf            [8,  N]
    k = Wk @ xf            [8,  N]
    v = Wv @ xf            [C,  N]
    score[n, m] = q[:, n] . k[:, m]
    P = softmax(score, axis=n)          (per-column softmax)
    out[c, m] = sum_n v[c, n] P[n, m]
    att = gamma * out + xf

Kernel strategy (8 cores = 4 batches x 2 column-halves of N):
  - Score via the rank-8 q^T k form with 4x PE row tiling: q and k are
    projected on device into partition groups {0,32,64,96} (one matmul with
    zero-padded replicated weights), so each 128-row score tile issues four
    concurrent 512-column matmuls on distinct 32-row PE groups.
  - exp() is split between ScalarE (exact activation) and VectorE using a
    Schraudolph fast-exp: bits16 = trunc(A*s + B) written as int16 is the
    bf16 bit pattern of ~exp(s) (+-3%, zero-mean after calibrating B; the
    softmax normalization cancels the scale, final rel err ~4e-5).
  - V@E accumulates with a ones-row appended to V^T so one PSUM chain gives
    both gamma*(V @ E) (gamma folded into Wv on the host) and colsum(E).
  - 1/colsum via exp(-ln(colsum)) on ScalarE; broadcast on GpSimd; residual
    add uses the exact f32 input.
"""

import numpy as np

import concourse.bass as bass
import concourse.bacc as bacc
import concourse.tile as tile
from concourse import mybir
from concourse.bass_utils import run_bass_kernel_spmd

# Problem shape (hardcoded per contract).
B, C, H, W = 4, 64, 64, 64
N = H * W           # 4096
MHALF = N // 2      # 2048 columns of the score/output handled per core
NT = N // 128       # 32 row-tiles of the score matrix
N_CORES = 8

F32 = mybir.dt.float32
BF16 = mybir.dt.bfloat16
I16 = mybir.dt.int16
_NP_BF16 = mybir.dt.np(BF16)

# Schraudolph fast-exp constants for bf16 bit patterns (DVE truncates on
# f32->int16 convert; B calibrated offline on the actual score distribution).
FEXP_A = 184.6650390625   # 2^7 / ln 2
FEXP_B = 16249.5

_PROGRAM = None


def _scalar_owns_exp(t: int, h: int) -> bool:
    """Split the exp tiles between ScalarE (h=0) and VectorE (h=1) so the
    two engines always run concurrently within an iteration."""
    return h == 0


def _build_program() -> bass.Bass:
    nc = bacc.Bacc()

    # xfp is host-permuted to [xk | other-half]: the n-order only permutes
    # the V@E accumulation, so the k-projection reads xfp[:, 0:MHALF]
    xfp_d = nc.declare_dram_parameter("xfp", [C, N], BF16, isOutput=False)
    xkf_d = nc.declare_dram_parameter("xkf", [C, MHALF], F32, isOutput=False)
    # packed weights: wq4 [64,128] | wk4 [64,128] | wv^T*gamma/2 [64,64];
    # wpk2 is the partition 64-127 replica with wq4/wk4 zeroed so full-array
    # projection matmuls contract exactly once
    wpk_d = nc.declare_dram_parameter("wpk", [C, 320], BF16, isOutput=False)
    wpk2_d = nc.declare_dram_parameter("wpk2", [C, 320], BF16, isOutput=False)
    out_d = nc.declare_dram_parameter("out", [C, MHALF], F32, isOutput=True)

    EXP = mybir.ActivationFunctionType.Exp
    LN = mybir.ActivationFunctionType.Ln
    MULT = mybir.AluOpType.mult
    ADD = mybir.AluOpType.add

    from concourse.hw_specs import get_activation_tables

    act_sets = list(get_activation_tables(nc.m.arch))
    nle_id = act_sets.index("natural_log_exp_and_others")

    with TileCtx(nc) as (tc, sing, epool, apool, psS, psO):
        # ---- input loads: few large DMAs (each DMA trigger costs ~600ns of
        # queue time); weights+xkp on the scalar queue feed the k/q
        # projections first, bulk xfp/xkf on the sync queue ----
        wpk_sb = sing.tile([128, 320], BF16, name="wpk_sb")
        nc.scalar.dma_start(out=wpk_sb[0:64, :], in_=wpk_d[:, :])
        nc.scalar.dma_start(out=wpk_sb[64:128, :], in_=wpk2_d[:, :])
        wq4_sb = wpk_sb[:, 0:128]
        wk4_sb = wpk_sb[:, 128:256]
        wv_sb = wpk_sb[:, 256:320]
        xfp_sb = sing.tile([128, N], BF16, name="xfp_sb")
        # first chunk is the k-projection input: smallest possible critical path
        nc.sync.dma_start(out=xfp_sb[0:64, 0:MHALF], in_=xfp_d[:, 0:MHALF])
        nc.sync.dma_start(out=xfp_sb[0:64, MHALF:N], in_=xfp_d[:, MHALF:N])
        nc.sync.dma_start(out=xfp_sb[64:128, :], in_=xfp_d[:, :])
        xkf_sb = sing.tile([C, MHALF], F32, name="xkf_sb")
        # activation tables are first needed by exp(0); emit the load after
        # the DMA triggers so it does not delay the weight/xkp transfers
        nc.scalar.add_instruction(
            mybir.InstLoadActFuncSet(
                name=nc.get_next_instruction_name(),
                act_func_set_id=nle_id,
                ins=[],
                outs=[],
            )
        )

        # ---- k4 = Wk-projection of this core's half (partition group 0;
        # k4 rows 8+ are zero via the rep1 weight layout) ----
        k4_sb = sing.tile([128, MHALF], BF16, name="k4_sb")
        for i in range(2):
            kp = psS.tile([128, 1024], F32, tag="S", name="kp")
            for cc in range(2):
                lo = i * 1024 + cc * 512
                nc.tensor.matmul(
                    kp[:, cc * 512 : (cc + 1) * 512],
                    lhsT=wk4_sb[0:64, :],
                    rhs=xfp_sb[0:64, lo : lo + 512],
                    start=True,
                    stop=True,
                    tile_position=(0, 0),
                )
            if i == 0:
                nc.scalar.copy(out=k4_sb[:, 0:1024], in_=kp)
            else:
                nc.vector.tensor_copy(out=k4_sb[:, 1024:2048], in_=kp)

        # ---- q4 = Wq-projection: chunk 0 in the prologue (group 0, fed by
        # the first xfp DMA); chunks 1-3 emitted inside the loop as
        # full-array matmuls (wq4 rows 64-127 zero), needed from t = 8*qi ----
        q4_sb = sing.tile([128, N], BF16, name="q4_sb")

        def emit_q_chunk(qi):
            qp = psS.tile([128, 1024], F32, tag="S", name="qp")
            for cc in range(2):
                lo = qi * 1024 + cc * 512
                if qi == 0:
                    nc.tensor.matmul(
                        qp[:, cc * 512 : (cc + 1) * 512],
                        lhsT=wq4_sb[0:64, :],
                        rhs=xfp_sb[0:64, lo : lo + 512],
                        start=True,
                        stop=True,
                        tile_position=(0, 0),
                    )
                else:
                    nc.tensor.matmul(
                        qp[:, cc * 512 : (cc + 1) * 512],
                        lhsT=wq4_sb,
                        rhs=xfp_sb[:, lo : lo + 512],
                        start=True,
                        stop=True,
                    )
            qsl = slice(qi * 1024, (qi + 1) * 1024)
            if qi % 2 == 0:
                nc.scalar.copy(out=q4_sb[:, qsl], in_=qp)
            else:
                nc.vector.tensor_copy(out=q4_sb[:, qsl], in_=qp)

        emit_q_chunk(0)

        # ---- vaugT[n, 0:64] = (gamma*Wv @ xf)^T tile, vaugT[n, 64] = 1 ----
        vaug_sb = sing.tile([128, NT * 65], BF16, name="vaug_sb")
        vaug3 = vaug_sb.rearrange("p (t u) -> p t u", u=65)
        nc.vector.memset(vaug3[:, :, 64:65], 1.0)

        def emit_vt_chunk(vv):
            # chunk 0: group-0 matmuls on wv/2 with a x2 copy (first xfp DMA
            # only); later chunks: full-array (both replicas x wv/2 sum
            # exactly), costing no PE config switch inside the loop
            vtp = psS.tile([128, 512], F32, tag="S", name="vtp")
            for i in range(8):
                t = vv * 8 + i
                if vv == 0:
                    nc.tensor.matmul(
                        vtp[:, i * 64 : (i + 1) * 64],
                        lhsT=xfp_sb[0:64, t * 128 : (t + 1) * 128],
                        rhs=wv_sb[0:64, :],
                        start=True,
                        stop=True,
                        tile_position=(0, 0),
                    )
                else:
                    nc.tensor.matmul(
                        vtp[:, i * 64 : (i + 1) * 64],
                        lhsT=xfp_sb[:, t * 128 : (t + 1) * 128],
                        rhs=wv_sb,
                        start=True,
                        stop=True,
                    )
            dst = vaug3[:, vv * 8 : (vv + 1) * 8, 0:64]
            src = vtp.rearrange("p (i u) -> p i u", u=64)
            if vv == 0:
                nc.vector.tensor_scalar_mul(dst, src, 2.0)
            else:
                nc.vector.tensor_copy(out=dst, in_=src)

        # ---- main loop, software-pipelined: emit score(t) and exp(t), then
        # V@E(t-1), so the PE streams score(t)+V@E(t-1) back-to-back while
        # the Scalar/Vector engines exp() the previous tile pair ----
        O_ps = psO.tile([65, MHALF], F32, name="O_ps")

        def emit_score_exp(t):
            Es = []
            for h in range(2):
                S = psS.tile([128, 1024], F32, tag="S", name="S_ps")
                for cc in range(2):
                    r = 2 * h + cc
                    # full-array matmul: k4 is zero outside rows 0:8, so the
                    # 128-partition contraction picks out q-group 0 exactly
                    # (alternating PE tile configs costs ~400ns per switch)
                    nc.tensor.matmul(
                        S[:, cc * 512 : (cc + 1) * 512],
                        lhsT=q4_sb[:, t * 128 : (t + 1) * 128],
                        rhs=k4_sb[:, r * 512 : (r + 1) * 512],
                        start=True,
                        stop=True,
                    )
                if _scalar_owns_exp(t, h):
                    E = epool.tile([128, 1024], BF16, tag="E", name="E_sb")
                    nc.scalar.activation(out=E, in_=S, func=EXP)
                    Es.append(E)
                else:
                    E = epool.tile([128, 1024], I16, tag="E", name="Ei_sb")
                    nc.vector.tensor_scalar(E, S, FEXP_A, FEXP_B, MULT, ADD)
                    Es.append(E.bitcast(BF16))
            return Es

        def emit_ve(t, Es):
            va_t = vaug3[:, t, :]
            for h in range(2):
                for cc in range(2):
                    r = 2 * h + cc
                    nc.tensor.matmul(
                        O_ps[:, r * 512 : (r + 1) * 512],
                        lhsT=va_t,
                        rhs=Es[h][:, cc * 512 : (cc + 1) * 512],
                        start=(t == 0),
                        stop=(t == NT - 1),
                    )

        prev_Es = emit_score_exp(0)
        emit_vt_chunk(0)
        for t in range(1, NT):
            Es = emit_score_exp(t)
            if t in (1, 3, 5):
                emit_q_chunk((t + 1) // 2)
            if t in (2, 4, 6):
                emit_vt_chunk(t // 2)
            if t == 8:
                # residual input is only needed by the tail; the memset makes
                # the DMA wait for mid-loop instead of competing for HBM
                # bandwidth with the prologue loads
                nc.vector.memset(xkf_sb[0:1, 0:1], 0.0)
                nc.sync.dma_start(out=xkf_sb, in_=xkf_d[:, :])
            emit_ve(t - 1, prev_Es)
            prev_Es = Es
        emit_ve(NT - 1, prev_Es)

        # ---- normalize + residual, store. Tile serializes same-PSUM-tile
        # readers in EMISSION order, so emit all O_ps colsum reads (LN) before
        # the first O_ps[0:C] read (MUL); EXP/broadcast interleave between ----
        lnts, rcps, bcss = [], [], []
        for j in range(4):
            sl = slice(j * 512, (j + 1) * 512)
            lnt = apool.tile([1, 512], F32, tag="lnt", name="lnt")
            nc.scalar.activation(out=lnt, in_=O_ps[64:65, sl], func=LN)
            lnts.append(lnt)
        for j in range(4):
            rcp = apool.tile([1, 512], BF16, tag="rcp", name="rcp")
            nc.scalar.activation(out=rcp, in_=lnts[j], func=EXP, scale=-1.0)
            rcps.append(rcp)
            bcs = apool.tile([C, 512], BF16, tag="bcs", name="bcs")
            nc.gpsimd.partition_broadcast(bcs, rcp)
            bcss.append(bcs)
        for j in range(4):
            sl = slice(j * 512, (j + 1) * 512)
            tmp = apool.tile([C, 512], F32, tag="tmp", name="tmp")
            nc.vector.tensor_mul(tmp, O_ps[0:C, sl], bcss[j])
            att = apool.tile([C, 512], F32, tag="att", name="att")
            nc.vector.tensor_add(att, tmp, xkf_sb[:, sl])
            nc.sync.dma_start(out=out_d[:, sl], in_=att)

    nc.finalize()
    return nc


class TileCtx:
    """TileContext plus the tile pools used by the kernel."""

    def __init__(self, nc: bass.Bass):
        self.nc = nc

    def __enter__(self):
        from contextlib import ExitStack

        self._stack = ExitStack()
        tc = self._stack.enter_context(tile.TileContext(self.nc))
        sing = self._stack.enter_context(tc.tile_pool(name="sing", bufs=1))
        epool = self._stack.enter_context(tc.tile_pool(name="epool", bufs=6))
        apool = self._stack.enter_context(tc.tile_pool(name="apool", bufs=4))
        psS = self._stack.enter_context(tc.tile_pool(name="psS", bufs=2, space="PSUM"))
        psO = self._stack.enter_context(tc.tile_pool(name="psO", bufs=1, space="PSUM"))
        return tc, sing, epool, apool, psS, psO

    def __exit__(self, *exc):
        return self._stack.__exit__(*exc)


def get_program() -> bass.Bass:
    global _PROGRAM
    if _PROGRAM is None:
        _PROGRAM = _build_program()
    return _PROGRAM


def make_in_maps(x, Wq, Wk, Wv, gamma):
    """Shard the full inputs into per-core input maps (host-side prep only:
    reshape/slice, replicated zero-padded weight layouts, cast to bf16)."""
    x = np.ascontiguousarray(np.asarray(x, dtype=np.float32))
    Wq = np.asarray(Wq, dtype=np.float32)
    Wk = np.asarray(Wk, dtype=np.float32)
    Wv = np.asarray(Wv, dtype=np.float32)
    gamma = float(np.asarray(gamma, dtype=np.float32).reshape(()))

    def rep4(Wm):  # [8, 64] -> [64, 128] with W^T at free-cols 32a..32a+8
        out = np.zeros((C, 128), dtype=_NP_BF16)
        for a in range(4):
            out[:, 32 * a : 32 * a + 8] = Wm.T.astype(_NP_BF16)
        return out

    def rep1(Wm):  # [8, 64] -> [64, 128] with W^T only at free-cols 0..8
        out = np.zeros((C, 128), dtype=_NP_BF16)
        out[:, 0:8] = Wm.T.astype(_NP_BF16)
        return out

    # wv is halved: the v-projection contracts over both 64-row xfp replicas
    wvh = (0.5 * gamma * Wv.T).astype(_NP_BF16)
    wpk = np.ascontiguousarray(
        np.concatenate([rep4(Wq), rep1(Wk), wvh], axis=1)
    )  # [64, 320]
    wpk2 = np.ascontiguousarray(
        np.concatenate([np.zeros((C, 256), dtype=_NP_BF16), wvh], axis=1)
    )

    in_maps = []
    for core in range(N_CORES):
        b, h = divmod(core, 2)
        xf = x[b].reshape(C, N)
        xk = xf[:, h * MHALF : (h + 1) * MHALF]
        xo = xf[:, (1 - h) * MHALF : (2 - h) * MHALF]
        in_maps.append(
            {
                # n-permuted so this core's key half leads (see kernel docs)
                "xfp": np.ascontiguousarray(
                    np.concatenate([xk, xo], axis=1).astype(_NP_BF16)
                ),
                "xkf": np.ascontiguousarray(xk),
                "wpk": wpk,
                "wpk2": wpk2,
            }
        )
    return in_maps


def gather(results):
    out = np.empty((B, C, N), dtype=np.float32)
    for core in range(N_CORES):
        b, h = divmod(core, 2)
        out[b][:, h * MHALF : (h + 1) * MHALF] = results[core]["out"]
    return out.reshape(B, C, H, W)


def run(inputs, **spmd_kwargs):
    nc = get_program()
    in_maps = make_in_maps(
        inputs["x"], inputs["Wq"], inputs["Wk"], inputs["Wv"], inputs["gamma"]
    )
    res = run_bass_kernel_spmd(nc, in_maps, core_ids=list(range(N_CORES)), **spmd_kwargs)
    return gather(res.results), res


def kernel(x, Wq, Wk, Wv, gamma):
    out, _ = run({"x": x, "Wq": Wq, "Wk": Wk, "Wv": Wv, "gamma": gamma})
    return out


# revision 47
# speedup vs baseline: 1.0809x; 1.0008x over previous
"""Trainium2 Bass kernel for nn_Attention_62362925138174.

Reference computation (per batch b, with xf = x[b].reshape(C, N), N = H*W):
    q = Wq @ xf            [8,  N]
    k = Wk @ xf            [8,  N]
    v = Wv @ xf            [C,  N]
    score[n, m] = q[:, n] . k[:, m]
    P = softmax(score, axis=n)          (per-column softmax)
    out[c, m] = sum_n v[c, n] P[n, m]
    att = gamma * out + xf

Kernel strategy (8 cores = 4 batches x 2 column-halves of N):
  - Score via the rank-8 q^T k form with 4x PE row tiling: q and k are
    projected on device into partition groups {0,32,64,96} (one matmul with
    zero-padded replicated weights), so each 128-row score tile issues four
    concurrent 512-column matmuls on distinct 32-row PE groups.
  - exp() is split between ScalarE (exact activation) and VectorE using a
    Schraudolph fast-exp: bits16 = trunc(A*s + B) written as int16 is the
    bf16 bit pattern of ~exp(s) (+-3%, zero-mean after calibrating B; the
    softmax normalization cancels the scale, final rel err ~4e-5).
  - V@E accumulates with a ones-row appended to V^T so one PSUM chain gives
    both gamma*(V @ E) (gamma folded into Wv on the host) and colsum(E).
  - 1/colsum via exp(-ln(colsum)) on ScalarE; broadcast on GpSimd; residual
    add uses the exact f32 input.
"""

import numpy as np

import concourse.bass as bass
import concourse.bacc as bacc
import concourse.tile as tile
from concourse import mybir
from concourse.bass_utils import run_bass_kernel_spmd

# Problem shape (hardcoded per contract).
B, C, H, W = 4, 64, 64, 64
N = H * W           # 4096
MHALF = N // 2      # 2048 columns of the score/output handled per core
NT = N // 128       # 32 row-tiles of the score matrix
N_CORES = 8

F32 = mybir.dt.float32
BF16 = mybir.dt.bfloat16
I16 = mybir.dt.int16
I8 = mybir.dt.int8
F8E4 = mybir.dt.float8e4
_NP_BF16 = mybir.dt.np(BF16)

# Schraudolph fast-exp constants for fp8e4 bit patterns (DVE truncates on
# f32->int8 convert; B8/BETA calibrated offline on the actual scores).
# ScalarE's exact exp carries the same 2^(~1/8) scale via its bias so the
# two engines' E tiles stay consistent (softmax cancels common scale).
FEXP8_A = 11.541560327111707   # 2^3 / ln 2
FEXP8_B = 57.0
EXP_BIAS = 0.0866
VSCALE = 8.0                   # vaug fp8 values are 8*gamma*v; rcp divides out

_PROGRAM = None


def _scalar_owns_exp(t: int, h: int) -> bool:
    """Split the exp tiles between ScalarE (h=0) and VectorE (h=1) so the
    two engines always run concurrently within an iteration."""
    return h == 0


def _build_program() -> bass.Bass:
    nc = bacc.Bacc()

    # xfp is host-permuted to [xk | other-half]: the n-order only permutes
    # the V@E accumulation, so the k-projection reads xfp[:, 0:MHALF]
    xfp_d = nc.declare_dram_parameter("xfp", [C, N], BF16, isOutput=False)
    xkf_d = nc.declare_dram_parameter("xkf", [C, MHALF], F32, isOutput=False)
    # packed weights: wq4 [64,128] | wk4 [64,128] | wv^T*gamma/2 [64,64];
    # wpk2 is the partition 64-127 replica with wq4/wk4 zeroed so full-array
    # projection matmuls contract exactly once
    wpk_d = nc.declare_dram_parameter("wpk", [C, 320], BF16, isOutput=False)
    wpk2_d = nc.declare_dram_parameter("wpk2", [C, 320], BF16, isOutput=False)
    out_d = nc.declare_dram_parameter("out", [C, MHALF], F32, isOutput=True)

    EXP = mybir.ActivationFunctionType.Exp
    LN = mybir.ActivationFunctionType.Ln
    MULT = mybir.AluOpType.mult
    ADD = mybir.AluOpType.add

    from concourse.hw_specs import get_activation_tables

    act_sets = list(get_activation_tables(nc.m.arch))
    nle_id = act_sets.index("natural_log_exp_and_others")

    with TileCtx(nc) as (tc, sing, epool, apool, psS, psO):
        # ---- input loads: few large DMAs (each DMA trigger costs ~600ns of
        # queue time); weights+xkp on the scalar queue feed the k/q
        # projections first, bulk xfp/xkf on the sync queue ----
        wpk_sb = sing.tile([128, 320], BF16, name="wpk_sb")
        nc.scalar.dma_start(out=wpk_sb[0:64, :], in_=wpk_d[:, :])
        nc.scalar.dma_start(out=wpk_sb[64:128, :], in_=wpk2_d[:, :])
        wq4_sb = wpk_sb[:, 0:128]
        wk4_sb = wpk_sb[:, 128:256]
        wv_sb = wpk_sb[:, 256:320]
        xfp_sb = sing.tile([128, N], BF16, name="xfp_sb")
        # first chunk is the k-projection input: smallest possible critical path
        nc.sync.dma_start(out=xfp_sb[0:64, 0:MHALF], in_=xfp_d[:, 0:MHALF])
        nc.sync.dma_start(out=xfp_sb[0:64, MHALF:N], in_=xfp_d[:, MHALF:N])
        nc.sync.dma_start(out=xfp_sb[64:128, :], in_=xfp_d[:, :])
        xkf_sb = sing.tile([C, MHALF], F32, name="xkf_sb")
        # bias constants for the scalar exp (scale-match with fp8 fastexp)
        # and the rcp (divides out VSCALE)
        bexp_sb = sing.tile([128, 1], F32, name="bexp_sb")
        nc.gpsimd.memset(bexp_sb, EXP_BIAS)
        brcp_sb = sing.tile([1, 1], F32, name="brcp_sb")
        nc.gpsimd.memset(brcp_sb, -2.0794415416798357)
        # activation tables are first needed by exp(0); emit the load after
        # the DMA triggers so it does not delay the weight/xkp transfers
        nc.scalar.add_instruction(
            mybir.InstLoadActFuncSet(
                name=nc.get_next_instruction_name(),
                act_func_set_id=nle_id,
                ins=[],
                outs=[],
            )
        )

        # ---- k4 = Wk-projection of this core's half (partition group 0;
        # k4 rows 8+ are zero via the rep1 weight layout) ----
        k4_sb = sing.tile([128, MHALF], BF16, name="k4_sb")
        for i in range(2):
            kp = psS.tile([128, 1024], F32, tag="S", name="kp")
            for cc in range(2):
                lo = i * 1024 + cc * 512
                nc.tensor.matmul(
                    kp[:, cc * 512 : (cc + 1) * 512],
                    lhsT=wk4_sb[0:64, :],
                    rhs=xfp_sb[0:64, lo : lo + 512],
                    start=True,
                    stop=True,
                    tile_position=(0, 0),
                )
            if i == 0:
                nc.scalar.copy(out=k4_sb[:, 0:1024], in_=kp)
            else:
                nc.vector.tensor_copy(out=k4_sb[:, 1024:2048], in_=kp)

        # ---- q4 = Wq-projection: chunk 0 in the prologue (group 0, fed by
        # the first xfp DMA); chunks 1-3 emitted inside the loop as
        # full-array matmuls (wq4 rows 64-127 zero), needed from t = 8*qi ----
        q4_sb = sing.tile([128, N], BF16, name="q4_sb")

        def emit_q_chunk(qi):
            qp = psS.tile([128, 1024], F32, tag="S", name="qp")
            for cc in range(2):
                lo = qi * 1024 + cc * 512
                if qi == 0:
                    nc.tensor.matmul(
                        qp[:, cc * 512 : (cc + 1) * 512],
                        lhsT=wq4_sb[0:64, :],
                        rhs=xfp_sb[0:64, lo : lo + 512],
                        start=True,
                        stop=True,
                        tile_position=(0, 0),
                    )
                else:
                    nc.tensor.matmul(
                        qp[:, cc * 512 : (cc + 1) * 512],
                        lhsT=wq4_sb,
                        rhs=xfp_sb[:, lo : lo + 512],
                        start=True,
                        stop=True,
                    )
            qsl = slice(qi * 1024, (qi + 1) * 1024)
            if qi % 2 == 0:
                nc.scalar.copy(out=q4_sb[:, qsl], in_=qp)
            else:
                nc.vector.tensor_copy(out=q4_sb[:, qsl], in_=qp)

        emit_q_chunk(0)

        # ---- vaugT[n, 0:64] = 8*(gamma*Wv @ xf)^T tile, vaugT[n, 64] = 1;
        # fp8 with row stride 80 (DoubleRow weight APs need step%16==0) ----
        vaug_sb = sing.tile([128, NT * 80], F8E4, name="vaug_sb")
        vaug3 = vaug_sb.rearrange("p (t u) -> p t u", u=80)
        nc.vector.memset(vaug3[:, :, 64:65], 1.0)

        def emit_vt_chunk(vv):
            # chunk 0: group-0 matmuls on wv/2 with a x2 copy (first xfp DMA
            # only); later chunks: full-array (both replicas x wv/2 sum
            # exactly), costing no PE config switch inside the loop
            vtp = psS.tile([128, 512], F32, tag="S", name="vtp")
            for i in range(8):
                t = vv * 8 + i
                if vv == 0:
                    nc.tensor.matmul(
                        vtp[:, i * 64 : (i + 1) * 64],
                        lhsT=xfp_sb[0:64, t * 128 : (t + 1) * 128],
                        rhs=wv_sb[0:64, :],
                        start=True,
                        stop=True,
                        tile_position=(0, 0),
                    )
                else:
                    nc.tensor.matmul(
                        vtp[:, i * 64 : (i + 1) * 64],
                        lhsT=xfp_sb[:, t * 128 : (t + 1) * 128],
                        rhs=wv_sb,
                        start=True,
                        stop=True,
                    )
            dst = vaug3[:, vv * 8 : (vv + 1) * 8, 0:64]
            src = vtp.rearrange("p (i u) -> p i u", u=64)
            nc.vector.tensor_scalar_mul(dst, src, 2.0 * VSCALE if vv == 0 else VSCALE)

        # ---- main loop, software-pipelined: E tiles for a t-PAIR are packed
        # [E_t0 | E_t1] so V@E runs fp8 DoubleRow matmuls (2 contraction
        # tiles per pass); the pair's V@E is emitted after the NEXT pair's
        # first score so the PE never waits on the exp engines ----
        O_ps = psO.tile([65, MHALF], F32, name="O_ps")
        NP2 = NT // 2

        def emit_score_exp(t, pairE):
            j = t % 2
            for h in range(2):
                S = psS.tile([128, 1024], F32, tag="S", name="S_ps")
                for cc in range(2):
                    r = 2 * h + cc
                    # full-array matmul: k4 is zero outside rows 0:8, so the
                    # 128-partition contraction picks out q-group 0 exactly
                    # (alternating PE tile configs costs ~400ns per switch)
                    nc.tensor.matmul(
                        S[:, cc * 512 : (cc + 1) * 512],
                        lhsT=q4_sb[:, t * 128 : (t + 1) * 128],
                        rhs=k4_sb[:, r * 512 : (r + 1) * 512],
                        start=True,
                        stop=True,
                    )
                dst = pairE[h][:, j * 1024 : (j + 1) * 1024]
                if h == 0:
                    nc.scalar.activation(out=dst, in_=S, func=EXP, bias=bexp_sb)
                else:
                    nc.vector.tensor_scalar(dst, S, FEXP8_A, FEXP8_B, MULT, ADD)

        def alloc_pair():
            E0 = epool.tile([128, 2048], F8E4, tag="E", name="E_sb")
            E1 = epool.tile([128, 2048], I8, tag="E", name="Ei_sb")
            return (E0, E1)

        def emit_ve_pair(p, pairE):
            va = vaug3[:, 2 * p : 2 * p + 2, 0:65]
            r0 = pairE[0].rearrange("p (j m) -> p j m", j=2)
            r1 = pairE[1].bitcast(F8E4).rearrange("p (j m) -> p j m", j=2)
            for r in range(4):
                h, cc = r // 2, r % 2
                rhs = (r0 if h == 0 else r1)[:, :, cc * 512 : (cc + 1) * 512]
                nc.tensor.matmul(
                    O_ps[:, r * 512 : (r + 1) * 512],
                    lhsT=va,
                    rhs=rhs,
                    start=(p == 0),
                    stop=(p == NP2 - 1),
                    perf_mode=mybir.MatmulPerfMode.DoubleRow,
                )

        pairE = alloc_pair()
        emit_score_exp(0, pairE)
        emit_vt_chunk(0)
        prev_pair = None
        for t in range(1, NT):
            if t % 2 == 0:
                prev_prev, prev_pair, pairE = prev_pair, pairE, alloc_pair()
            emit_score_exp(t, pairE)
            if t in (1, 3, 5):
                emit_q_chunk((t + 1) // 2)
            if t in (2, 4, 6):
                emit_vt_chunk(t // 2)
            if t == 8:
                # residual input is only needed by the tail; the memset makes
                # the DMA wait for mid-loop instead of competing for HBM
                # bandwidth with the prologue loads
                nc.vector.memset(xkf_sb[0:1, 0:1], 0.0)
                nc.sync.dma_start(out=xkf_sb, in_=xkf_d[:, :])
            if t % 2 == 0 and prev_prev is not None:
                emit_ve_pair(t // 2 - 2, prev_prev)
        emit_ve_pair(NP2 - 2, prev_pair)
        emit_ve_pair(NP2 - 1, pairE)

        # ---- normalize + residual, store. Tile serializes same-PSUM-tile
        # readers in EMISSION order, so emit all O_ps colsum reads (LN) before
        # the first O_ps[0:C] read (MUL); EXP/broadcast interleave between ----
        lnts, rcps, bcss = [], [], []
        for j in range(4):
            sl = slice(j * 512, (j + 1) * 512)
            lnt = apool.tile([1, 512], F32, tag="lnt", name="lnt")
            nc.scalar.activation(out=lnt, in_=O_ps[64:65, sl], func=LN)
            lnts.append(lnt)
        for j in range(4):
            rcp = apool.tile([1, 512], BF16, tag="rcp", name="rcp")
            # divide out the VSCALE folded into vaug
            nc.scalar.activation(
                out=rcp, in_=lnts[j], func=EXP, scale=-1.0, bias=brcp_sb
            )
            rcps.append(rcp)
            bcs = apool.tile([C, 512], BF16, tag="bcs", name="bcs")
            nc.gpsimd.partition_broadcast(bcs, rcp)
            bcss.append(bcs)
        for j in range(4):
            sl = slice(j * 512, (j + 1) * 512)
            tmp = apool.tile([C, 512], F32, tag="tmp", name="tmp")
            nc.vector.tensor_mul(tmp, O_ps[0:C, sl], bcss[j])
            att = apool.tile([C, 512], F32, tag="att", name="att")
            nc.vector.tensor_add(att, tmp, xkf_sb[:, sl])
            nc.sync.dma_start(out=out_d[:, sl], in_=att)

    nc.finalize()
    return nc


class TileCtx:
    """TileContext plus the tile pools used by the kernel."""

    def __init__(self, nc: bass.Bass):
        self.nc = nc

    def __enter__(self):
        from contextlib import ExitStack

        self._stack = ExitStack()
        tc = self._stack.enter_context(tile.TileContext(self.nc))
        sing = self._stack.enter_context(tc.tile_pool(name="sing", bufs=1))
        epool = self._stack.enter_context(tc.tile_pool(name="epool", bufs=6))
        apool = self._stack.enter_context(tc.tile_pool(name="apool", bufs=4))
        psS = self._stack.enter_context(tc.tile_pool(name="psS", bufs=2, space="PSUM"))
        psO = self._stack.enter_context(tc.tile_pool(name="psO", bufs=1, space="PSUM"))
        return tc, sing, epool, apool, psS, psO

    def __exit__(self, *exc):
        return self._stack.__exit__(*exc)


def get_program() -> bass.Bass:
    global _PROGRAM
    if _PROGRAM is None:
        _PROGRAM = _build_program()
    return _PROGRAM


def make_in_maps(x, Wq, Wk, Wv, gamma):
    """Shard the full inputs into per-core input maps (host-side prep only:
    reshape/slice, replicated zero-padded weight layouts, cast to bf16)."""
    x = np.ascontiguousarray(np.asarray(x, dtype=np.float32))
    Wq = np.asarray(Wq, dtype=np.float32)
    Wk = np.asarray(Wk, dtype=np.float32)
    Wv = np.asarray(Wv, dtype=np.float32)
    gamma = float(np.asarray(gamma, dtype=np.float32).reshape(()))

    def rep4(Wm):  # [8, 64] -> [64, 128] with W^T at free-cols 32a..32a+8
        out = np.zeros((C, 128), dtype=_NP_BF16)
        for a in range(4):
            out[:, 32 * a : 32 * a + 8] = Wm.T.astype(_NP_BF16)
        return out

    def rep1(Wm):  # [8, 64] -> [64, 128] with W^T only at free-cols 0..8
        out = np.zeros((C, 128), dtype=_NP_BF16)
        out[:, 0:8] = Wm.T.astype(_NP_BF16)
        return out

    # wv is halved: the v-projection contracts over both 64-row xfp replicas
    wvh = (0.5 * gamma * Wv.T).astype(_NP_BF16)
    wpk = np.ascontiguousarray(
        np.concatenate([rep4(Wq), rep1(Wk), wvh], axis=1)
    )  # [64, 320]
    wpk2 = np.ascontiguousarray(
        np.concatenate([np.zeros((C, 256), dtype=_NP_BF16), wvh], axis=1)
    )

    in_maps = []
    for core in range(N_CORES):
        b, h = divmod(core, 2)
        xf = x[b].reshape(C, N)
        xk = xf[:, h * MHALF : (h + 1) * MHALF]
        xo = xf[:, (1 - h) * MHALF : (2 - h) * MHALF]
        in_maps.append(
            {
                # n-permuted so this core's key half leads (see kernel docs)
                "xfp": np.ascontiguousarray(
                    np.concatenate([xk, xo], axis=1).astype(_NP_BF16)
                ),
                "xkf": np.ascontiguousarray(xk),
                "wpk": wpk,
                "wpk2": wpk2,
            }
        )
    return in_maps


def gather(results):
    out = np.empty((B, C, N), dtype=np.float32)
    for core in range(N_CORES):
        b, h = divmod(core, 2)
        out[b][:, h * MHALF : (h + 1) * MHALF] = results[core]["out"]
    return out.reshape(B, C, H, W)


def run(inputs, **spmd_kwargs):
    nc = get_program()
    in_maps = make_in_maps(
        inputs["x"], inputs["Wq"], inputs["Wk"], inputs["Wv"], inputs["gamma"]
    )
    res = run_bass_kernel_spmd(nc, in_maps, core_ids=list(range(N_CORES)), **spmd_kwargs)
    return gather(res.results), res


def kernel(x, Wq, Wk, Wv, gamma):
    out, _ = run({"x": x, "Wq": Wq, "Wk": Wk, "Wv": Wv, "gamma": gamma})
    return out
